# revision 3
# baseline (speedup 1.0000x reference)
"""RBF Gram kernel K[i,j] = exp(-||x_i - y_j||^2) on 8 Trainium2 cores.

Sharding: rows of x (and of the output) split 8 ways; y replicated.
Per core: out[1024, 8192] = exp(2*(x@y^T) - x2[:,None] - y2[None,:]).

Device math per [128n x 512m] tile (all in one PSUM accumulation group):
    psum = xh^T yh + xh^T yl + xl^T yh      (bf16 split of x^T, y^T; err ~7e-4)
         + ones2^T r2                       (r2 = bf16 hi/lo split of -y2/2)
    out  = Exp(2*psum + bias), bias = -x2 per-partition  (ScalarE, one op)

exp(-sq) with sq >= ~85 underflows f32 to denormals; ACT's Exp produces
correct denormals down to arg ~ -97.3 (measured), matching the reference.
"""

import numpy as np
import ml_dtypes

import concourse.bass as bass
import concourse.bacc as bacc
import concourse.mybir as mybir
import concourse.tile as tile
from concourse.bass_utils import run_bass_kernel_spmd

F32 = mybir.dt.float32
BF16 = mybir.dt.bfloat16
BF = ml_dtypes.bfloat16

N = 8192          # rows of x / output
M = 8192          # rows of y / output cols
D = 128           # feature dim = contraction = partition dim
NCORES = 8
NS = N // NCORES  # 1024 output rows per core
NBLK = NS // 128  # 8 n-blocks per core
MGRP = 2048       # columns per PSUM group (4 banks)
NGRP = M // MGRP  # 4 groups
SUB = 512         # matmul moving size (1 PSUM bank fp32)

_cached = {}


def _build_nc():
    nc = bacc.Bacc(None)

    yth = nc.dram_tensor("yth", [D, M], BF16, kind="ExternalInput")
    ytl = nc.dram_tensor("ytl", [D, M], BF16, kind="ExternalInput")
    xth = nc.dram_tensor("xth", [D, NS], BF16, kind="ExternalInput")
    xtl = nc.dram_tensor("xtl", [D, NS], BF16, kind="ExternalInput")
    r2 = nc.dram_tensor("r2", [2, M], BF16, kind="ExternalInput")
    nb = nc.dram_tensor("nb", [128, NBLK], F32, kind="ExternalInput")
    out = nc.dram_tensor("out", [NS, M], F32, kind="ExternalOutput")

    with tile.TileContext(nc) as tc:
        with (
            tc.tile_pool(name="cst", bufs=1) as cst,
            tc.tile_pool(name="outp", bufs=4) as outp,
            tc.tile_pool(name="ps", bufs=2, space="PSUM") as ps,
        ):
            yth_t = cst.tile([D, M], BF16, tag="yth")
            ytl_t = cst.tile([D, M], BF16, tag="ytl")
            xth_t = cst.tile([D, NS], BF16, tag="xth")
            xtl_t = cst.tile([D, NS], BF16, tag="xtl")
            r2_t = cst.tile([2, M], BF16, tag="r2")
            nb_t = cst.tile([128, NBLK], F32, tag="nb")
            on2_t = cst.tile([2, 128], BF16, tag="on2")
            nc.sync.dma_start(xth_t[:], xth[:])
            nc.sync.dma_start(xtl_t[:], xtl[:])
            nc.sync.dma_start(yth_t[:], yth[:])
            nc.sync.dma_start(ytl_t[:], ytl[:])
            nc.sync.dma_start(r2_t[:], r2[:])
            nc.sync.dma_start(nb_t[:], nb[:])
            nc.vector.memset(on2_t[:], 1.0)

            for bi in range(NBLK):
                xh_b = xth_t[:, bi * 128:(bi + 1) * 128]
                xl_b = xtl_t[:, bi * 128:(bi + 1) * 128]
                for g in range(NGRP):
                    p = ps.tile([128, MGRP], F32, tag="p")
                    # weight-reuse order: all subtiles per stationary operand
                    for s in range(MGRP // SUB):
                        m0 = g * MGRP + s * SUB
                        nc.tensor.matmul(
                            p[:, s * SUB:(s + 1) * SUB], xh_b,
                            yth_t[:, m0:m0 + SUB], start=True, stop=False)
                        nc.tensor.matmul(
                            p[:, s * SUB:(s + 1) * SUB], xh_b,
                            ytl_t[:, m0:m0 + SUB], start=False, stop=False)
                    for s in range(MGRP // SUB):
                        m0 = g * MGRP + s * SUB
                        nc.tensor.matmul(
                            p[:, s * SUB:(s + 1) * SUB], xl_b,
                            yth_t[:, m0:m0 + SUB], start=False, stop=False)
                    for s in range(MGRP // SUB):
                        m0 = g * MGRP + s * SUB
                        nc.tensor.matmul(
                            p[:, s * SUB:(s + 1) * SUB], on2_t[:],
                            r2_t[:, m0:m0 + SUB], start=False, stop=True)
                    o = outp.tile([128, MGRP], F32, tag="o")
                    nc.scalar.activation(
                        o[:], p[:], mybir.ActivationFunctionType.Exp,
                        bias=nb_t[:, bi:bi + 1], scale=2.0)
                    nc.sync.dma_start(
                        out[bi * 128:(bi + 1) * 128, g * MGRP:(g + 1) * MGRP],
                        o[:])

    nc.finalize()
    return nc


def _prep_in_maps(x, y):
    x = np.ascontiguousarray(np.asarray(x, dtype=np.float32))
    y = np.ascontiguousarray(np.asarray(y, dtype=np.float32))
    assert x.shape == (N, D) and y.shape == (M, D)

    # host prep (O(N*D), trivial): transposes, bf16 hi/lo splits, norms
    xt = x.T.astype(np.float32)                     # [D, N]
    yt = y.T.astype(np.float32)                     # [D, M]
    xth_f = xt.astype(BF)
    xtl_f = (xt - xth_f.astype(np.float32)).astype(BF)
    yth_f = yt.astype(BF)
    ytl_f = (yt - yth_f.astype(np.float32)).astype(BF)
    x2 = np.einsum("nd,nd->n", x, x, dtype=np.float64).astype(np.float32)
    y2 = np.einsum("md,md->m", y, y, dtype=np.float64).astype(np.float32)
    rh = (-0.5 * y2).astype(np.float32)
    r2h = rh.astype(BF)
    r2l = (rh - r2h.astype(np.float32)).astype(BF)
    r2_v = np.stack([r2h, r2l], axis=0)             # [2, M]

    in_maps = []
    for c in range(NCORES):
        sl = slice(c * NS, (c + 1) * NS)
        nb_v = -x2[sl].reshape(NBLK, 128).T.copy()  # [128, NBLK]
        in_maps.append({
            "yth": np.ascontiguousarray(yth_f),
            "ytl": np.ascontiguousarray(ytl_f),
            "xth": np.ascontiguousarray(xth_f[:, sl]),
            "xtl": np.ascontiguousarray(xtl_f[:, sl]),
            "r2": np.ascontiguousarray(r2_v),
            "nb": nb_v,
        })
    return in_maps


def kernel(x, y):
    if "nc" not in _cached:
        _cached["nc"] = _build_nc()
    nc = _cached["nc"]
    in_maps = _prep_in_maps(x, y)
    res = run_bass_kernel_spmd(nc, in_maps, core_ids=list(range(NCORES)))
    return np.concatenate([r["out"] for r in res.results], axis=0)


def run_traced(inputs):
    """Profiled run; returns BassKernelResults (exec_time_ns etc.)."""
    if "nc" not in _cached:
        _cached["nc"] = _build_nc()
    nc = _cached["nc"]
    in_maps = _prep_in_maps(**inputs)
    return run_bass_kernel_spmd(
        nc, in_maps, core_ids=list(range(NCORES)), trace=True)
